# revision 18
# baseline (speedup 1.0000x reference)
"""Trainium2 Bass kernel: DSOT graph builder (Sinkhorn + top-k edges).

Full inputs: x (16,1024,384) f32, pos (1024,2) f32.
Sharding: data-parallel over B; 2 batches per core on 8 cores.

Math (per batch):
  xn = normalize(x); G'_ij = xn_i.xn_j + 0.2 pos_i.pos_j - h_j  (h=0.1|pos|^2)
  cost = 1 + h_i + h_j - (G'_ij + h_j);  mean over matrix via colsum trick
  K = exp(-cost/(EPS*mean));  K~ = fp16(2^6 K) has EXACT diagonal 64.0
  Sinkhorn (K symmetric): w_{s+1} = r/(K w_s), 40 half-steps; u20=w39, v20=w40
    matvec on PE: m = K~ @ W~ (weights = fp16(2^6 w)); z = m + D where
    D = 2^12 w - 2^6 W~ corrects the fp16 rounding of w through the unit diag.
  Selection surrogate M = K~ * (u+v)_j (row factors do not affect order);
  top-16 via max8/max_index/match_replace; vals renormalized exactly.
"""

import math
import numpy as np

B, N, D = 16, 1024, 384
EPS = 0.1
LAMBDA_POS = 0.1
ALPHA = 0.2
KTOP = 16
NCORES = 8
BPC = B // NCORES  # batches per core
RT = N // 128      # row tiles
HALF_STEPS = 40

LN64 = math.log(64.0)

_COMPILED = None


def _build():
    import concourse.bass as bass
    import concourse.bacc as bacc
    import concourse.mybir as mybir
    from concourse.tile import TileContext
    from concourse.masks import make_identity
    from concourse import bass_isa

    f32 = mybir.dt.float32
    f16 = mybir.dt.float16
    u32 = mybir.dt.uint32
    AX = mybir.AxisListType
    AF = mybir.ActivationFunctionType

    nc = bacc.Bacc("TRN2", target_bir_lowering=False, debug=False)

    x2 = nc.declare_dram_parameter("x2", [BPC, N, D], f32, isOutput=False)
    bf16 = mybir.dt.bfloat16
    auxLH_d = nc.declare_dram_parameter("auxLH", [3, N], bf16, isOutput=False)
    auxLL_d = nc.declare_dram_parameter("auxLL", [3, N], bf16, isOutput=False)
    auxRH_d = nc.declare_dram_parameter("auxRH", [3, N], bf16, isOutput=False)
    auxRL_d = nc.declare_dram_parameter("auxRL", [3, N], bf16, isOutput=False)
    h1p_d = nc.declare_dram_parameter("h1p", [128, RT], f32, isOutput=False)
    c0e_d = nc.declare_dram_parameter("c0e", [128, 1], f32, isOutput=False)
    ov = nc.declare_dram_parameter("ov", [BPC, N, KTOP], f32, isOutput=True)
    dbg = nc.declare_dram_parameter("dbg", [128, 41], f32, isOutput=True)
    ab_d = nc.dram_tensor("ab_scratch", [N], mybir.dt.float32)
    m_d = nc.dram_tensor("m_scratch", [2, N], mybir.dt.float32)
    oi = nc.declare_dram_parameter("oi", [BPC, N, KTOP], u32, isOutput=True)

    with TileContext(nc) as tc:
        with (
            tc.tile_pool(name="persist", bufs=1) as persist,
            tc.tile_pool(name="xload", bufs=3) as xload,
            tc.tile_pool(name="xnTp", bufs=1) as xnTp,
            tc.tile_pool(name="gsb", bufs=1) as gsbp,
            tc.tile_pool(name="ktil", bufs=1) as ktilp,
            tc.tile_pool(name="small", bufs=4) as small,
            tc.tile_pool(name="sink", bufs=3) as sink,
            tc.tile_pool(name="mrow", bufs=2) as mrowp,
            tc.tile_pool(name="outs", bufs=3) as outsp,
            tc.tile_pool(name="gpsum", bufs=2, space="PSUM") as gpsum,
            tc.tile_pool(name="trpsum", bufs=2, space="PSUM") as trpsum,
            tc.tile_pool(name="mpsum", bufs=2, space="PSUM") as mpsum,
        ):
            ident = persist.tile([128, 128], mybir.dt.bfloat16, tag="ident")
            make_identity(nc, ident)
            ones128 = persist.tile([128, 1], f32, tag="ones128")
            nc.vector.memset(ones128, 1.0)
            ones1x = persist.tile([1, 128], f32, tag="ones1x")
            nc.vector.memset(ones1x, 1.0)
            h1p = persist.tile([128, RT], f32, tag="h1p")
            nc.sync.dma_start(out=h1p, in_=h1p_d[:, :])
            c0e = persist.tile([128, 1], f32, tag="c0e")
            nc.sync.dma_start(out=c0e, in_=c0e_d[:, :])
            bf = mybir.dt.bfloat16
            aLH = persist.tile([3, N], bf, tag="aLH")
            nc.sync.dma_start(out=aLH, in_=auxLH_d[:, :])
            aLL = persist.tile([3, N], bf, tag="aLL")
            nc.sync.dma_start(out=aLL, in_=auxLL_d[:, :])
            aRH = persist.tile([3, N], bf, tag="aRH")
            nc.sync.dma_start(out=aRH, in_=auxRH_d[:, :])
            aRL = persist.tile([3, N], bf, tag="aRL")
            nc.sync.dma_start(out=aRL, in_=auxRL_d[:, :])

            # fp16 K~ tiles, persistent per batch
            Ktil = [
                [ktilp.tile([128, N], f16, tag=f"K_{b}_{rt}", name=f"K_{b}_{rt}") for rt in range(RT)]
                for b in range(BPC)
            ]
            q39 = [persist.tile([128, RT], f32, tag=f"q39_{b}", name=f"q39_{b}") for b in range(BPC)]
            q40 = [persist.tile([128, RT], f32, tag=f"q40_{b}", name=f"q40_{b}") for b in range(BPC)]

            def build_K(b):
                """normalize -> bf16 hi/lo transpose -> G (3-term) -> gamma"""
                bfl = mybir.dt.bfloat16
                XhT = [xnTp.tile([128, N], bfl, tag=f"XhT{kc}", name=f"XhT{kc}")
                       for kc in range(3)]
                XlT = [xnTp.tile([128, N], bfl, tag=f"XlT{kc}", name=f"XlT{kc}")
                       for kc in range(3)]
                xall = xload.tile([128, RT, D], f32, tag="xall", bufs=1)
                nc.sync.dma_start(
                    out=xall, in_=x2[b].rearrange("(c p) d -> p c d", p=128))
                for rt in range(RT):
                    xt = xall[:, rt, :]
                    sq = xload.tile([128, D], f32, tag="sq")
                    ss = small.tile([128, 1], f32, tag="ss")
                    nc.scalar.activation(out=sq, in_=xt, func=AF.Square, accum_out=ss)
                    sr = small.tile([128, 1], f32, tag="sr")
                    nc.scalar.activation(out=sr, in_=ss, func=AF.Sqrt)
                    rn = small.tile([128, 1], f32, tag="rn")
                    nc.vector.reciprocal(rn, sr)
                    xn = xload.tile([128, D], f32, tag="xn")
                    nc.scalar.activation(out=xn, in_=xt, func=AF.Copy, scale=rn)
                    xh = xload.tile([128, D], bfl, tag="xh")
                    nc.scalar.activation(out=xh, in_=xn, func=AF.Copy)
                    xl = xload.tile([128, D], bfl, tag="xl")
                    nc.vector.tensor_sub(xl, xn, xh)
                    for kc in range(3):
                        for tsrc, tdst in ((xh, XhT), (xl, XlT)):
                            tp = trpsum.tile([128, 128], f32, tag="tp")
                            nc.tensor.matmul(tp, tsrc[:, kc * 128:(kc + 1) * 128],
                                             ident, start=True, stop=True)
                            nc.scalar.activation(
                                out=tdst[kc][:, rt * 128:(rt + 1) * 128], in_=tp,
                                func=AF.Copy)

                Gsb = [gsbp.tile([128, N], f32, tag=f"G{rt}", name=f"G{rt}") for rt in range(RT)]
                rs = small.tile([128, 2 * RT], f32, tag="rs")
                for rt in range(RT):
                    for nh in range(2):
                        gp = gpsum.tile([128, 512], f32, tag="gp")
                        mm = []
                        for kc in range(3):
                            mm += [(XhT[kc], XhT[kc]), (XhT[kc], XlT[kc]),
                                   (XlT[kc], XhT[kc])]
                        mm += [(aLH, aRH), (aLH, aRL), (aLL, aRH)]
                        for i, (lt, rtm) in enumerate(mm):
                            nc.tensor.matmul(
                                gp,
                                lt[:, rt * 128:(rt + 1) * 128],
                                rtm[:, nh * 512:(nh + 1) * 512],
                                start=(i == 0), stop=(i == len(mm) - 1))
                        nc.scalar.activation(
                            out=Gsb[rt][:, nh * 512:(nh + 1) * 512], in_=gp,
                            func=AF.Copy, accum_out=rs[:, rt * 2 + nh:rt * 2 + nh + 1])
                # total sum -> gamma (per-partition, via gpsimd all-reduce)
                rsum = small.tile([128, 1], f32, tag="rsum")
                nc.vector.reduce_sum(rsum, rs, axis=AX.X)
                tot = small.tile([128, 1], f32, tag="tot")
                nc.gpsimd.partition_all_reduce(tot, rsum, channels=128,
                                               reduce_op=bass_isa.ReduceOp.add)
                t1 = small.tile([128, 1], f32, tag="t1")
                nc.vector.tensor_scalar(t1, tot, -EPS / float(N * N), None,
                                        op0=mybir.AluOpType.mult)
                t2 = small.tile([128, 1], f32, tag="t2")
                nc.vector.tensor_add(t2, t1, c0e)
                gbc = small.tile([128, 1], f32, tag="gbc")
                nc.vector.reciprocal(gbc, t2)
                bias8 = small.tile([128, RT], f32, tag="bias8")
                nc.vector.tensor_mul(bias8, h1p, gbc.to_broadcast([128, RT]))
                nc.vector.tensor_scalar(bias8, bias8, -1.0, LN64,
                                        op0=mybir.AluOpType.mult,
                                        op1=mybir.AluOpType.add)
                for rt in range(RT):
                    nc.scalar.activation(
                        out=Ktil[b][rt], in_=Gsb[rt], func=AF.Exp,
                        bias=bias8[:, rt:rt + 1], scale=gbc)
                if b == 0:
                    dt_ = outsp.tile([128, 40], f32, tag="dbg0")
                    nc.vector.tensor_copy(dt_[:, 0:1], gbc)
                    nc.vector.tensor_copy(dt_[:, 1:9], bias8)
                    nc.vector.tensor_copy(dt_[:, 9:10], tot)
                    nc.vector.tensor_copy(dt_[:, 10:18], Ktil[b][0][:, 0:8])
                    nc.vector.tensor_copy(dt_[:, 18:26], Gsb[0][:, 0:8])
                    nc.vector.tensor_copy(dt_[:, 26:27], rsum)
                    nc.sync.dma_start(out=dbg[:, 0:27], in_=dt_[:, 0:27])

            def sinkhorn(b):
                Wt = sink.tile([128, RT], f16, tag="W")
                nc.vector.memset(Wt, 64.0)
                Dt = sink.tile([128, RT], f32, tag="Dt")
                nc.vector.memset(Dt, 0.0)
                for s in range(HALF_STEPS):
                    mp = mpsum.tile([1, N], f32, tag="mp")
                    for nh in range(2):
                        for kc in range(RT):
                            nc.tensor.matmul(
                                mp[:, nh * 512:(nh + 1) * 512],
                                Wt[:, kc:kc + 1],
                                Ktil[b][kc][:, nh * 512:(nh + 1) * 512],
                                start=(kc == 0), stop=(kc == RT - 1))
                    msb = sink.tile([1, N], f32, tag="msb")
                    nc.scalar.activation(out=msb, in_=mp, func=AF.Copy)
                    nc.sync.dma_start(out=m_d[b:b+1, :], in_=msb)
                    mres = sink.tile([128, RT], f32, tag="mres")
                    nc.sync.dma_start(
                        out=mres, in_=m_d[b:b+1, :].rearrange("o (c p) -> (o p) c", p=128))
                    z = sink.tile([128, RT], f32, tag="z")
                    nc.vector.tensor_add(z, mres, Dt)
                    if s == HALF_STEPS - 2:
                        q = q39[b]
                    elif s == HALF_STEPS - 1:
                        q = q40[b]
                    else:
                        q = sink.tile([128, RT], f32, tag="q")
                    nc.vector.reciprocal(q, z)
                    if b == 0 and s == 0:
                        dt2 = outsp.tile([128, 14], f32, tag="dbg1")
                        nc.vector.tensor_copy(dt2[:, 0:4], mres[:, 0:4])
                        nc.vector.tensor_copy(dt2[:, 4:8], z[:, 0:4])
                        nc.vector.tensor_copy(dt2[:, 8:12], q[:, 0:4])
                        nc.vector.tensor_copy(dt2[:, 12:14], Dt[:, 0:2])
                        nc.sync.dma_start(out=dbg[:, 27:41], in_=dt2)
                    if s < HALF_STEPS - 1:
                        Wt = sink.tile([128, RT], f16, tag="W")
                        nc.scalar.activation(out=Wt, in_=q, func=AF.Copy, scale=256.0)
                        s14 = sink.tile([128, RT], f32, tag="s14")
                        nc.scalar.activation(out=s14, in_=q, func=AF.Copy,
                                             scale=16384.0)
                        W6 = sink.tile([128, RT], f16, tag="W6")
                        nc.scalar.activation(out=W6, in_=q, func=AF.Copy,
                                             scale=16384.0)
                        Dt = sink.tile([128, RT], f32, tag="Dt")
                        nc.vector.tensor_sub(Dt, s14, W6)

            def topk(b):
                absum = small.tile([128, RT], f32, tag="absum")
                nc.vector.tensor_add(absum, q39[b], q40[b])
                q39_8 = small.tile([128, RT], f32, tag="q39_8")
                nc.vector.tensor_scalar(q39_8, q39[b], 0.125, None,
                                        op0=mybir.AluOpType.mult)
                nc.sync.dma_start(
                    out=ab_d.rearrange("(c p) -> p c", p=128), in_=absum)
                abp = mrowp.tile([128, N], f32, tag="abp", bufs=1)
                ab_bc = bass.AP(tensor=ab_d, offset=0,
                                ap=[[0, 128], [1, N]])
                nc.gpsimd.dma_start(out=abp, in_=ab_bc)
                for rt in range(RT):
                    Mt = mrowp.tile([128, N], f32, tag="Mt")
                    nc.vector.tensor_mul(Mt, Ktil[b][rt], abp)
                    v16 = outsp.tile([128, KTOP], f32, tag="v16")
                    i16 = outsp.tile([128, KTOP], u32, tag="i16")
                    nc.vector.max(v16[:, 0:8], Mt)
                    nc.vector.max_index(i16[:, 0:8], v16[:, 0:8], Mt)
                    nc.vector.match_replace(Mt, in_to_replace=v16[:, 0:8],
                                            in_values=Mt, imm_value=0.0)
                    nc.vector.max(v16[:, 8:16], Mt)
                    nc.vector.max_index(i16[:, 8:16], v16[:, 8:16], Mt)
                    # exact values + normalization
                    t1 = small.tile([128, 1], f32, tag="tk1")
                    nc.vector.tensor_mul(t1, q39[b][:, rt:rt + 1],
                                         q40[b][:, rt:rt + 1])
                    dg = small.tile([128, 1], f32, tag="dg")
                    nc.vector.tensor_scalar(dg, t1, 16.0, ALPHA,
                                            op0=mybir.AluOpType.mult,
                                            op1=mybir.AluOpType.add)
                    s15 = small.tile([128, 1], f32, tag="s15")
                    nc.vector.reduce_sum(s15, v16[:, 1:KTOP], axis=AX.X)
                    t3 = small.tile([128, 1], f32, tag="tk3")
                    nc.vector.tensor_mul(t3, q39_8[:, rt:rt + 1], s15)
                    den = small.tile([128, 1], f32, tag="den")
                    nc.vector.tensor_add(den, t3, dg)
                    rec = small.tile([128, 1], f32, tag="rec")
                    nc.vector.reciprocal(rec, den)
                    vout = outsp.tile([128, KTOP], f32, tag="vout")
                    nc.scalar.activation(out=vout[:, 0:1], in_=dg, func=AF.Copy,
                                         scale=rec)
                    pf = small.tile([128, 1], f32, tag="pf")
                    nc.vector.tensor_mul(pf, q39_8[:, rt:rt + 1], rec)
                    nc.scalar.activation(out=vout[:, 1:KTOP], in_=v16[:, 1:KTOP],
                                         func=AF.Copy, scale=pf)
                    nc.sync.dma_start(out=ov[b, rt * 128:(rt + 1) * 128, :],
                                      in_=vout)
                    nc.sync.dma_start(out=oi[b, rt * 128:(rt + 1) * 128, :],
                                      in_=i16)

            # emission order chosen so batch-1 PE phases overlap batch-0 DVE topk
            build_K(0)
            sinkhorn(0)
            build_K(1)
            topk(0)
            sinkhorn(1)
            topk(1)

    nc.finalize()
    return nc


def _prep_host(pos):
    import ml_dtypes
    bf16 = ml_dtypes.bfloat16
    pos = np.asarray(pos, np.float32)
    a2 = (pos * pos).sum(-1)
    h = (LAMBDA_POS * a2).astype(np.float32)
    sp = np.sqrt(np.float32(2 * LAMBDA_POS))
    pT = (sp * pos.T).astype(np.float32)          # (2, N)
    aux3L = np.concatenate([pT, np.ones((1, N), np.float32)], 0)
    aux3R = np.concatenate([pT, -h[None, :]], 0).astype(np.float32)
    LH = aux3L.astype(bf16)
    LL = (aux3L - LH.astype(np.float32)).astype(bf16)
    RH = aux3R.astype(bf16)
    RL = (aux3R - RH.astype(np.float32)).astype(bf16)
    h1p = (1.0 + h).reshape(RT, 128).T.copy().astype(np.float32)
    hbar = float(h.mean())
    c0e = np.full((128, 1), EPS * (1.0 + 2.0 * hbar + 1e-8), np.float32)
    return LH, LL, RH, RL, h1p, c0e


def kernel(x, pos):
    global _COMPILED
    from concourse.bass_utils import run_bass_kernel_spmd

    x = np.asarray(x, np.float32)
    pos = np.asarray(pos, np.float32)
    if _COMPILED is None:
        _COMPILED = _build()
    nc = _COMPILED

    LH, LL, RH, RL, h1p, c0e = _prep_host(pos)
    in_maps = []
    for c in range(NCORES):
        in_maps.append({
            "x2": np.ascontiguousarray(x[c * BPC:(c + 1) * BPC]),
            "auxLH": LH, "auxLL": LL, "auxRH": RH, "auxRL": RL,
            "h1p": h1p, "c0e": c0e,
        })
    res = run_bass_kernel_spmd(nc, in_maps, list(range(NCORES)))
    vals = np.zeros((B, N, KTOP), np.float32)
    idx = np.zeros((B, N, KTOP), np.int64)
    for c in range(NCORES):
        vals[c * BPC:(c + 1) * BPC] = res.results[c]["ov"]
        idx[c * BPC:(c + 1) * BPC] = res.results[c]["oi"].astype(np.int64)

    offs = (np.arange(B, dtype=np.int64) * N)[:, None, None]
    src = np.broadcast_to(np.arange(N, dtype=np.int64)[None, :, None],
                          (B, N, KTOP)) + offs
    dst = idx + offs
    edge_index = np.stack([src.reshape(-1), dst.reshape(-1)], 0).astype(np.int32)
    edge_weight = vals.reshape(-1).astype(np.float32)
    batch = np.repeat(np.arange(B, dtype=np.int32), N)
    return edge_index, edge_weight, batch
